# revision 6
# baseline (speedup 1.0000x reference)
"""Bahdanau additive attention on 8 TRN2 NeuronCores.

  q = queries @ Wq.T            [B,H]
  k = keys @ Wk.T               [K,H]
  scores[b,k] = sum_h wv[h] * tanh(q[b,h] + k[k,h])
  out = softmax_k(scores) @ values

Sharding: data-parallel over B (128 queries per core); keys/values/weights
replicated. No collectives.

Per-core plan (H=128 on partitions):
  - transpose queries/Wq/Wk/keys via PE, project to qT [H,B] and kT [H,K]
  - per query b: one ScalarE instr  feat = tanh(kT + qT[:,b])  (fused
    per-partition bias), output in bf16
  - scores row b = wv.T @ feat via PE (bf16), written to PSUM partition b
  - exp via ScalarE with fused free-dim row-sum (accum_out); no max
    subtraction needed since |scores| <= ||wv||_1 ~ 9
  - attn^T via PE transposes, then 16 accumulating f32 matmuls vs values
  - scale by 1/rowsum, DMA out
"""

import sys

if "/opt/trn_rl_repo" not in sys.path:
    sys.path.insert(0, "/opt/trn_rl_repo")

import numpy as np

import concourse.bacc as bacc
import concourse.bass as bass
import concourse.mybir as mybir
import concourse.tile as tile
from concourse.masks import make_identity

B, K, H, D = 1024, 2048, 128, 512
N_CORES = 8
BS = B // N_CORES  # 128 queries per core
P = 128
DC = D // P   # 4 depth chunks
KT = K // P   # 16 key tiles of 128
KC = K // 512  # 4 chunks of 512 keys

F32 = mybir.dt.float32
BF16 = mybir.dt.bfloat16
Tanh = mybir.ActivationFunctionType.Tanh
Exp = mybir.ActivationFunctionType.Exp


def build_nc() -> bass.Bass:
    nc = bacc.Bacc()
    q_ext = nc.declare_dram_parameter("queries", [BS, D], F32, isOutput=False)
    k_ext = nc.declare_dram_parameter("keys", [K, D], F32, isOutput=False)
    v_ext = nc.declare_dram_parameter("values", [K, D], F32, isOutput=False)
    wq_ext = nc.declare_dram_parameter("Wq", [H, D], F32, isOutput=False)
    wk_ext = nc.declare_dram_parameter("Wk", [H, D], F32, isOutput=False)
    wv_ext = nc.declare_dram_parameter("wv", [H, 1], F32, isOutput=False)
    out_ext = nc.declare_dram_parameter("out", [BS, D], F32, isOutput=True)

    with tile.TileContext(nc) as tc:
        with (
            tc.tile_pool(name="consts", bufs=1) as consts,
            tc.tile_pool(name="big", bufs=1) as big,
            tc.tile_pool(name="stage", bufs=3) as stage,
        ):
            identity = consts.tile([P, P], F32)
            make_identity(nc, identity)

            wv_f32 = consts.tile([P, 1], F32)
            nc.sync.dma_start(wv_f32, wv_ext[:, :])
            wv_bf = consts.tile([P, 1], BF16)
            nc.vector.tensor_copy(wv_bf, wv_f32)

            wq_sb = consts.tile([P, D], F32)
            nc.sync.dma_start(wq_sb, wq_ext[:, :])
            wk_sb = consts.tile([P, D], F32)
            nc.sync.dma_start(wk_sb, wk_ext[:, :])
            q_sb = consts.tile([P, D], F32)
            nc.sync.dma_start(q_sb, q_ext[:, :])

            # values, needed only for the final matmul; loaded early to overlap
            v_sb = big.tile([P, KT, 512], F32)
            for t in range(KT):
                nc.sync.dma_start(v_sb[:, t, :], v_ext[t * P:(t + 1) * P, :])

            wqT = consts.tile([P, DC, P], F32)   # [d%128, dchunk, h]
            wkT = consts.tile([P, DC, P], F32)
            qT_d = consts.tile([P, DC, P], F32)  # [d%128, dchunk, b]
            keysT = big.tile([P, DC, K], F32)    # [d%128, dchunk, k]
            qT = consts.tile([P, BS], F32)       # [h, b]
            kT = big.tile([P, K], F32)           # [h, k]

            with (
                tc.tile_pool(name="tpsum", bufs=3, space="PSUM") as tpsum,
                tc.tile_pool(name="ppsum", bufs=2, space="PSUM") as ppsum,
            ):
                for src, dst in ((wq_sb, wqT), (wk_sb, wkT), (q_sb, qT_d)):
                    for c in range(DC):
                        pt = tpsum.tile([P, P], F32, tag="tp")
                        nc.tensor.transpose(pt, src[:, c * P:(c + 1) * P], identity)
                        nc.vector.tensor_copy(dst[:, c, :], pt)

                for t in range(KT):
                    ks = stage.tile([P, D], F32, tag="kstage")
                    nc.sync.dma_start(ks, k_ext[t * P:(t + 1) * P, :])
                    for c in range(DC):
                        pt = tpsum.tile([P, P], F32, tag="tp")
                        nc.tensor.transpose(pt, ks[:, c * P:(c + 1) * P], identity)
                        nc.vector.tensor_copy(keysT[:, c, t * P:(t + 1) * P], pt)

                pq = ppsum.tile([P, BS], F32, tag="pp")
                for c in range(DC):
                    nc.tensor.matmul(pq, wqT[:, c, :], qT_d[:, c, :],
                                     start=(c == 0), stop=(c == DC - 1))
                nc.vector.tensor_copy(qT, pq)

                for s in range(KC):
                    pk = ppsum.tile([P, 512], F32, tag="pp2")
                    for c in range(DC):
                        nc.tensor.matmul(pk, wkT[:, c, :],
                                         keysT[:, c, s * 512:(s + 1) * 512],
                                         start=(c == 0), stop=(c == DC - 1))
                    nc.vector.tensor_copy(kT[:, s * 512:(s + 1) * 512], pk)

            ones_sb = consts.tile([P, 1], F32)
            nc.vector.memset(ones_sb, 1.0)

            # scoresT[k, b] held as 4 PSUM tiles of [128, 4, 128] (1 bank each)
            attnT = big.tile([P, KT, P], F32)  # [k%128, ktile, b]
            with (
                tc.tile_pool(name="spsum", bufs=1, space="PSUM") as spsum,
                tc.tile_pool(name="feats", bufs=3) as feats,
            ):
                scT = [spsum.tile([P, 4, P], F32, tag=f"sc{i}", name=f"scT{i}")
                       for i in range(4)]
                for b in range(BS):
                    ft = feats.tile([P, K], BF16, tag="feat")
                    nc.scalar.activation(ft, kT, Tanh, bias=qT[:, b:b + 1])
                    for t in range(KT):
                        nc.tensor.matmul(scT[t // 4][:, t % 4, b:b + 1],
                                         ft[:, t * P:(t + 1) * P], wv_bf,
                                         start=True, stop=True)
                for t in range(KT):
                    nc.scalar.activation(attnT[:, t, :], scT[t // 4][:, t % 4, :],
                                         Exp)

            with (
                tc.tile_pool(name="opsum", bufs=1, space="PSUM") as opsum,
            ):
                outp = opsum.tile([P, D], F32)
                sums = opsum.tile([P, 1], F32, tag="sums")
                for t in range(KT):
                    nc.tensor.matmul(outp, attnT[:, t, :], v_sb[:, t, :],
                                     start=(t == 0), stop=(t == KT - 1))
                for t in range(KT):
                    nc.tensor.matmul(sums, attnT[:, t, :], ones_sb,
                                     start=(t == 0), stop=(t == KT - 1))
                rsum = consts.tile([P, 1], F32)
                nc.vector.reciprocal(rsum, sums)
                out_sb = stage.tile([P, D], F32, tag="osb")
                nc.vector.tensor_scalar_mul(out_sb, outp, rsum)
                nc.sync.dma_start(out_ext[:, :], out_sb)

    nc.compile()
    return nc


_NC_CACHE: dict = {}


def _get_nc() -> bass.Bass:
    if "nc" not in _NC_CACHE:
        _NC_CACHE["nc"] = build_nc()
    return _NC_CACHE["nc"]


def make_in_maps(inputs: dict) -> list[dict]:
    queries = np.ascontiguousarray(np.asarray(inputs["queries"], np.float32))
    keys = np.ascontiguousarray(np.asarray(inputs["keys"], np.float32))
    values = np.ascontiguousarray(np.asarray(inputs["values"], np.float32))
    Wq = np.ascontiguousarray(np.asarray(inputs["Wq"], np.float32))
    Wk = np.ascontiguousarray(np.asarray(inputs["Wk"], np.float32))
    wv = np.ascontiguousarray(np.asarray(inputs["wv"], np.float32).reshape(H, 1))
    return [
        {
            "queries": queries[c * BS:(c + 1) * BS],
            "keys": keys,
            "values": values,
            "Wq": Wq,
            "Wk": Wk,
            "wv": wv,
        }
        for c in range(N_CORES)
    ]


def run(inputs: dict, trace: bool = False):
    """Returns (full_output [B, D] f32, BassKernelResults)."""
    from concourse.bass_utils import run_bass_kernel_spmd

    nc = _get_nc()
    res = run_bass_kernel_spmd(nc, make_in_maps(inputs), list(range(N_CORES)),
                               trace=trace)
    out = np.concatenate(
        [np.asarray(res.results[i]["out"], np.float32) for i in range(N_CORES)],
        axis=0,
    )
    return out, res


def kernel(**inputs) -> np.ndarray:
    out, _ = run(inputs, trace=False)
    return out
